# revision 3
# baseline (speedup 1.0000x reference)
"""2-layer GCN (message passing) on 8 TRN2 NeuronCores.

Strategy: fold the symmetric GCN normalization into per-row scalings by
dinv = rsqrt(deg), so propagation becomes Q = (A+I)^T @ P' with P' the
dinv-scaled linear outputs.  The (A+I) operator is materialized on host as
dense per-core count shards (dst-sharded, exact in fp8), and the propagate
is a dense bf16 x fp8 matmul accumulated in fp32 PSUM.  Between layers the
node-feature shards are exchanged with an 8-core AllGather.

Per core c (owns dst nodes [1250c, 1250(c+1))):
  W-matmul   : P = x_c @ W + b (fp32, node-major PSUM), scale rows by dinv
  AllGather  : bf16 shard [1280,128] -> full P' [10240,128]
  A-matmul   : Q^T[feat, dst] = sum_k P'[k-chunk]^T-as-weights @ A[k, dst]
  epilogue   : h = relu(Q) * dinv (feature-major, fp32)
Final: L2 row-normalize via ones-matmul column sums, then @ Wc + bc.
"""

import sys

if "/opt/trn_rl_repo" not in sys.path:
    sys.path.insert(0, "/opt/trn_rl_repo")

import numpy as np
import ml_dtypes

N, E, D, H, C = 10000, 640000, 128, 128, 40
NC_ = 8                 # cores
NSH = N // NC_          # 1250 nodes per core
NCH = 10                # 128-row chunks per core shard (padded)
NPAD = NCH * 128        # 1280 padded shard rows
NFULL = NPAD * NC_      # 10240 padded global rows
KCH = NFULL // 128      # 80 source chunks
SLICES = [(0, 512), (512, 512), (1024, NSH - 1024)]
MJ_LAST = NSH - 9 * 128  # 98 real rows in the last chunk

_cache = {}


def _build():
    import concourse.bass as bass  # noqa: F401
    import concourse.bacc as bacc
    import concourse.mybir as mybir
    import concourse.tile as tile

    dt = mybir.dt
    F32, BF16, FP8 = dt.float32, dt.bfloat16, dt.float8e4
    AF = mybir.ActivationFunctionType
    RG = [list(range(NC_))]

    nc = bacc.Bacc("TRN2", target_bir_lowering=False, debug=False, num_devices=NC_)

    xT_d = nc.dram_tensor("xT", [128, NSH], F32, kind="ExternalInput").ap()
    A_d = nc.dram_tensor("A", [128, KCH * NSH], FP8, kind="ExternalInput").ap()
    degpp_d = nc.dram_tensor("degpp", [128, NCH], F32, kind="ExternalInput").ap()
    degrow_d = nc.dram_tensor("degrow", [1, NSH], F32, kind="ExternalInput").ap()
    W1_d = nc.dram_tensor("W1", [H, H], F32, kind="ExternalInput").ap()
    W2_d = nc.dram_tensor("W2", [H, H], F32, kind="ExternalInput").ap()
    Wc_d = nc.dram_tensor("Wc", [H, C], F32, kind="ExternalInput").ap()
    b1_d = nc.dram_tensor("b1", [1, H], F32, kind="ExternalInput").ap()
    b2_d = nc.dram_tensor("b2", [1, H], F32, kind="ExternalInput").ap()
    bc_d = nc.dram_tensor("bc", [1, C], F32, kind="ExternalInput").ap()
    out_d = nc.dram_tensor("out", [NPAD, C], F32, kind="ExternalOutput").ap()

    with tile.TileContext(nc) as tc:
        with (
            tc.tile_pool(name="cst", bufs=1) as cst,
            tc.tile_pool(name="wk", bufs=1) as wk,
            tc.tile_pool(name="hpp", bufs=1) as hpp,
            tc.tile_pool(name="aring", bufs=8) as arp,
            tc.tile_pool(name="pfm", bufs=1, space="PSUM") as pfm,
            tc.tile_pool(name="pnm", bufs=2, space="PSUM") as pnm,
            tc.tile_pool(name="pns", bufs=1, space="PSUM") as pns_p,
            tc.tile_pool(name="dram", bufs=2, space="DRAM") as dram,
        ):
            # ---- constants / inputs to SBUF ----
            xT = cst.tile([128, NSH], F32, tag="xT")
            nc.sync.dma_start(out=xT[:], in_=xT_d)
            W1s = cst.tile([H, H], F32, tag="W1")
            nc.sync.dma_start(out=W1s[:], in_=W1_d)
            W2s = cst.tile([H, H], F32, tag="W2")
            nc.sync.dma_start(out=W2s[:], in_=W2_d)
            Wcs = cst.tile([H, C], F32, tag="Wc")
            nc.sync.dma_start(out=Wcs[:], in_=Wc_d)
            b1s = cst.tile([1, H], F32, tag="b1")
            nc.sync.dma_start(out=b1s[:], in_=b1_d)
            b2s = cst.tile([1, H], F32, tag="b2")
            nc.sync.dma_start(out=b2s[:], in_=b2_d)
            bcs = cst.tile([1, C], F32, tag="bc")
            nc.sync.dma_start(out=bcs[:], in_=bc_d)
            degpp = wk.tile([128, NCH], F32, tag="degpp")
            nc.sync.dma_start(out=degpp[:], in_=degpp_d)
            degrow = wk.tile([1, NSH], F32, tag="degrow")
            nc.sync.dma_start(out=degrow[:], in_=degrow_d)

            ones_r = cst.tile([1, 128], F32, tag="ones_r")
            nc.vector.memset(ones_r[:], 1.0)
            ones_c = cst.tile([128, 1], F32, tag="ones_c")
            nc.vector.memset(ones_c[:], 1.0)

            # ---- dinv = 1/sqrt(deg) ----
            dinvpp = cst.tile([128, NCH], F32, tag="dinvpp")
            tmp_pp = wk.tile([128, NCH], F32, tag="tmp_pp")
            nc.scalar.sqrt(tmp_pp[:], degpp[:])
            nc.vector.reciprocal(dinvpp[:], tmp_pp[:])
            dinvrow = cst.tile([1, NSH], F32, tag="dinvrow")
            tmp_row = wk.tile([1, NSH], F32, tag="tmp_row")
            nc.scalar.sqrt(tmp_row[:], degrow[:])
            nc.vector.reciprocal(dinvrow[:], tmp_row[:])

            # dinv broadcast across partitions: rank-1 matmul ones x dinvrow
            dinvbc = cst.tile([128, NSH], F32, tag="dinvbc")
            psb0 = pfm.tile([128, NSH], F32, tag="fm")
            for o, n in SLICES:
                nc.tensor.matmul(
                    out=psb0[:, o : o + n], lhsT=ones_r[:, :],
                    rhs=dinvrow[:, o : o + n], start=True, stop=True,
                )
            nc.scalar.copy(dinvbc[:], psb0[:])

            def layer(inT, Ws, bs, h_tag):
                # W-matmul: P' rows for own nodes, node-major, bf16
                sh = wk.tile([128, NPAD], BF16, tag="sh")
                # zero the last chunk (covers the 30 pad rows); real rows are
                # overwritten by the j=9 activation below
                nc.vector.memset(sh[:, 9 * 128 : NPAD], 0.0)
                for j in range(NCH):
                    mj = 128 if j < 9 else MJ_LAST
                    pj = pnm.tile([128, H], F32, tag="nm")
                    nc.tensor.matmul(
                        out=pj[:mj, :], lhsT=inT[:, j * 128 : j * 128 + mj],
                        rhs=Ws[:], start=True, stop=False,
                    )
                    nc.tensor.matmul(
                        out=pj[:mj, :], lhsT=ones_r[:, :mj], rhs=bs[:],
                        start=False, stop=True,
                    )
                    nc.scalar.activation(
                        sh[:mj, j * 128 : (j + 1) * 128], pj[:mj, :],
                        AF.Copy, scale=dinvpp[:mj, j : j + 1],
                    )

                # AllGather shards
                bounce = dram.tile([NPAD, 128], BF16, tag="bounce")
                agf = dram.tile([NFULL, 128], BF16, tag="agf", addr_space="Shared")
                nc.sync.dma_start(
                    out=bounce[:].rearrange("(j p) f -> p j f", p=128),
                    in_=sh[:].rearrange("p (j f) -> p j f", f=128),
                )
                nc.gpsimd.collective_compute(
                    "AllGather", mybir.AluOpType.bypass, replica_groups=RG,
                    ins=[bounce.opt()], outs=[agf.opt()],
                )
                hp = hpp.tile([128, KCH * 128], BF16, tag="hp")
                nc.sync.dma_start(
                    out=hp[:].rearrange("p (k f) -> p k f", f=128),
                    in_=agf[:].rearrange("(k p) f -> p k f", p=128),
                )

                # A-matmul: Q^T[feat, dst] accumulated over 80 source chunks
                ps = pfm.tile([128, NSH], F32, tag="fm")
                for k in range(KCH):
                    at = arp.tile([128, NSH], FP8, tag="a")
                    nc.sync.dma_start(out=at[:], in_=A_d[:, k * NSH : (k + 1) * NSH])
                    for o, n in SLICES:
                        nc.tensor.matmul(
                            out=ps[:, o : o + n],
                            lhsT=hp[:, k * 128 : (k + 1) * 128],
                            rhs=at[:, o : o + n],
                            start=(k == 0), stop=(k == KCH - 1),
                        )
                rel = wk.tile([128, NSH], F32, tag="rel")
                nc.scalar.activation(rel[:], ps[:], AF.Relu)
                hT = wk.tile([128, NSH], F32, tag=h_tag)
                nc.vector.tensor_mul(hT[:], rel[:], dinvbc[:])
                return hT

            h1T = layer(xT, W1s, b1s, "h1T")
            h2T = layer(h1T, W2s, b2s, "h2T")

            # ---- L2 normalize columns (per node) ----
            sq = wk.tile([128, NSH], F32, tag="sq")
            nc.scalar.square(sq[:], h2T[:])
            pns = pns_p.tile([1, NSH], F32, tag="ns")
            for o, n in SLICES:
                nc.tensor.matmul(
                    out=pns[:, o : o + n], lhsT=ones_c[:, :],
                    rhs=sq[:, o : o + n], start=True, stop=True,
                )
            sr = wk.tile([1, NSH], F32, tag="sr")
            nc.scalar.sqrt(sr[:], pns[:])
            nc.vector.tensor_scalar_max(sr[:], sr[:], 1e-12)
            rr = wk.tile([1, NSH], F32, tag="rr")
            nc.vector.reciprocal(rr[:], sr[:])
            psb = pfm.tile([128, NSH], F32, tag="fm")
            for o, n in SLICES:
                nc.tensor.matmul(
                    out=psb[:, o : o + n], lhsT=ones_r[:, :],
                    rhs=rr[:, o : o + n], start=True, stop=True,
                )
            hn = wk.tile([128, NSH], F32, tag="hn")
            nc.vector.tensor_mul(hn[:], h2T[:], psb[:])

            # ---- classifier ----
            oc = wk.tile([128, NCH * C], F32, tag="oc")
            nc.vector.memset(oc[:, 9 * C : NCH * C], 0.0)
            for j in range(NCH):
                mj = 128 if j < 9 else MJ_LAST
                pc = pnm.tile([128, H], F32, tag="nm")
                nc.tensor.matmul(
                    out=pc[:mj, :C], lhsT=hn[:, j * 128 : j * 128 + mj],
                    rhs=Wcs[:], start=True, stop=False,
                )
                nc.tensor.matmul(
                    out=pc[:mj, :C], lhsT=ones_r[:, :mj], rhs=bcs[:],
                    start=False, stop=True,
                )
                nc.scalar.copy(oc[:mj, j * C : (j + 1) * C], pc[:mj, :C])
            nc.sync.dma_start(
                out=out_d.rearrange("(j p) c -> p j c", p=128),
                in_=oc[:].rearrange("p (j c) -> p j c", c=C),
            )

    nc.compile()
    return nc


def _prep(inputs):
    x = np.asarray(inputs["x"], np.float32)
    ei = np.asarray(inputs["edge_index"])
    src = ei[0].astype(np.int64)
    dst = ei[1].astype(np.int64)
    loops = np.arange(N, dtype=np.int64)
    s_all = np.concatenate([src, loops])
    d_all = np.concatenate([dst, loops])
    deg = np.bincount(d_all, minlength=N).astype(np.float32)
    gsrc = (s_all // NSH) * NPAD + (s_all % NSH)  # padded global row of source
    dcore = d_all // NSH
    dloc = d_all % NSH

    W1 = np.ascontiguousarray(np.asarray(inputs["W1"], np.float32))
    W2 = np.ascontiguousarray(np.asarray(inputs["W2"], np.float32))
    Wc = np.ascontiguousarray(np.asarray(inputs["Wc"], np.float32))
    b1 = np.asarray(inputs["b1"], np.float32).reshape(1, H)
    b2 = np.asarray(inputs["b2"], np.float32).reshape(1, H)
    bc = np.asarray(inputs["bc"], np.float32).reshape(1, C)

    in_maps = []
    for c in range(NC_):
        m = dcore == c
        flat = gsrc[m] * NSH + dloc[m]
        Ac = np.bincount(flat, minlength=NFULL * NSH).astype(np.float32)
        Ac = Ac.reshape(KCH, 128, NSH).transpose(1, 0, 2)
        Ac = np.ascontiguousarray(Ac).reshape(128, KCH * NSH)
        degc = deg[c * NSH : (c + 1) * NSH]
        degpp = np.concatenate([degc, np.ones(NPAD - NSH, np.float32)])
        degpp = np.ascontiguousarray(degpp.reshape(NCH, 128).T)
        in_maps.append({
            "xT": np.ascontiguousarray(x[c * NSH : (c + 1) * NSH].T),
            "A": Ac.astype(ml_dtypes.float8_e4m3),
            "degpp": degpp,
            "degrow": np.ascontiguousarray(degc.reshape(1, NSH)),
            "W1": W1, "W2": W2, "Wc": Wc,
            "b1": b1, "b2": b2, "bc": bc,
        })
    return in_maps


def run(inputs, **spmd_kwargs):
    from concourse import bass_utils

    if "nc" not in _cache:
        _cache["nc"] = _build()
    in_maps = _prep(inputs)
    res = bass_utils.run_bass_kernel_spmd(
        _cache["nc"], in_maps, core_ids=list(range(NC_)), **spmd_kwargs
    )
    out = np.concatenate(
        [np.asarray(res.results[c]["out"])[:NSH] for c in range(NC_)], axis=0
    )
    return out.astype(np.float32), res


def kernel(**inputs):
    out, _ = run(inputs)
    return out


# revision 6
# speedup vs baseline: 1.1949x; 1.1949x over previous
"""2-layer GCN (message passing) on 8 TRN2 NeuronCores.

Strategy: fold the symmetric GCN normalization into per-row scalings by
dinv = rsqrt(deg), so propagation becomes Q = (A+I)^T @ P' with P' the
dinv-scaled linear outputs.  The (A+I) operator is materialized on host as
dense per-core count shards (dst-sharded, exact in fp8), and the propagate
is a dense bf16 x fp8 matmul accumulated in fp32 PSUM.  Between layers the
node-feature shards are exchanged with an 8-core AllGather.

Per core c (owns dst nodes [1250c, 1250(c+1))):
  W-matmul   : P = x_c @ W + b (fp32, node-major PSUM), scale rows by dinv
  AllGather  : bf16 shard [1280,128] -> full P' [10240,128]
  A-matmul   : Q^T[feat, dst] = sum_k P'[k-chunk]^T-as-weights @ A[k, dst]
  epilogue   : h = relu(Q) * dinv (feature-major, fp32)
Final: L2 row-normalize via ones-matmul column sums, then @ Wc + bc.
"""

import sys

if "/opt/trn_rl_repo" not in sys.path:
    sys.path.insert(0, "/opt/trn_rl_repo")

import numpy as np
import ml_dtypes

N, E, D, H, C = 10000, 640000, 128, 128, 40
NC_ = 8                 # cores
NSH = N // NC_          # 1250 nodes per core
NCH = 10                # 128-row chunks per core shard (padded)
NPAD = NCH * 128        # 1280 padded shard rows
NFULL = NPAD * NC_      # 10240 padded global rows
KCH = NFULL // 128      # 80 source chunks
SLICES = [(0, 512), (512, 512), (1024, NSH - 1024)]
MJ_LAST = NSH - 9 * 128  # 98 real rows in the last chunk

_cache = {}


def _build():
    import concourse.bass as bass  # noqa: F401
    import concourse.bacc as bacc
    import concourse.mybir as mybir
    import concourse.tile as tile

    dt = mybir.dt
    F32, BF16, FP8 = dt.float32, dt.bfloat16, dt.float8e4
    AF = mybir.ActivationFunctionType
    RG = [list(range(NC_))]

    nc = bacc.Bacc("TRN2", target_bir_lowering=False, debug=False, num_devices=NC_)

    xT_d = nc.dram_tensor("xT", [128, NSH], F32, kind="ExternalInput").ap()
    A_d = nc.dram_tensor("A", [128, KCH * NSH], FP8, kind="ExternalInput").ap()
    degpp_d = nc.dram_tensor("degpp", [128, NCH], F32, kind="ExternalInput").ap()
    degrow_d = nc.dram_tensor("degrow", [1, NSH], F32, kind="ExternalInput").ap()
    W1_d = nc.dram_tensor("W1", [H, H], F32, kind="ExternalInput").ap()
    W2_d = nc.dram_tensor("W2", [H, H], F32, kind="ExternalInput").ap()
    Wc_d = nc.dram_tensor("Wc", [H, C], F32, kind="ExternalInput").ap()
    b1_d = nc.dram_tensor("b1", [1, H], F32, kind="ExternalInput").ap()
    b2_d = nc.dram_tensor("b2", [1, H], F32, kind="ExternalInput").ap()
    bc_d = nc.dram_tensor("bc", [1, C], F32, kind="ExternalInput").ap()
    out_d = nc.dram_tensor("out", [NPAD, C], F32, kind="ExternalOutput").ap()

    with tile.TileContext(nc) as tc:
        with (
            tc.tile_pool(name="cst", bufs=1) as cst,
            tc.tile_pool(name="wk", bufs=1) as wk,
            tc.tile_pool(name="hpp", bufs=1) as hpp,
            tc.tile_pool(name="pfm", bufs=1, space="PSUM") as pfm,
            tc.tile_pool(name="pnm", bufs=2, space="PSUM") as pnm,
            tc.tile_pool(name="pns", bufs=1, space="PSUM") as pns_p,
            tc.tile_pool(name="dram", bufs=2, space="DRAM") as dram,
        ):
            # ---- constants / inputs to SBUF ----
            xT = cst.tile([128, NSH], F32, tag="xT")
            nc.sync.dma_start(out=xT[:], in_=xT_d)
            # resident adjacency (fp8, exact counts); issued early so the
            # load overlaps the kernel-entry barrier + W1 phase
            asb = cst.tile([128, KCH * NSH], FP8, tag="A")
            nc.sync.dma_start(out=asb[:], in_=A_d)
            W1s = cst.tile([H, H], F32, tag="W1")
            nc.sync.dma_start(out=W1s[:], in_=W1_d)
            W2s = cst.tile([H, H], F32, tag="W2")
            nc.sync.dma_start(out=W2s[:], in_=W2_d)
            Wcs = cst.tile([H, C], F32, tag="Wc")
            nc.sync.dma_start(out=Wcs[:], in_=Wc_d)
            b1s = cst.tile([1, H], F32, tag="b1")
            nc.sync.dma_start(out=b1s[:], in_=b1_d)
            b2s = cst.tile([1, H], F32, tag="b2")
            nc.sync.dma_start(out=b2s[:], in_=b2_d)
            bcs = cst.tile([1, C], F32, tag="bc")
            nc.sync.dma_start(out=bcs[:], in_=bc_d)
            degpp = wk.tile([128, NCH], F32, tag="degpp")
            nc.sync.dma_start(out=degpp[:], in_=degpp_d)
            degrow = wk.tile([1, NSH], F32, tag="degrow")
            nc.sync.dma_start(out=degrow[:], in_=degrow_d)

            ones_r = cst.tile([1, 128], F32, tag="ones_r")
            nc.vector.memset(ones_r[:], 1.0)
            ones_c = cst.tile([128, 1], F32, tag="ones_c")
            nc.vector.memset(ones_c[:], 1.0)

            # ---- dinv = 1/sqrt(deg) ----
            dinvpp = cst.tile([128, NCH], F32, tag="dinvpp")
            tmp_pp = wk.tile([128, NCH], F32, tag="tmp_pp")
            nc.scalar.sqrt(tmp_pp[:], degpp[:])
            nc.vector.reciprocal(dinvpp[:], tmp_pp[:])
            dinvrow = cst.tile([1, NSH], F32, tag="dinvrow")
            tmp_row = wk.tile([1, NSH], F32, tag="tmp_row")
            nc.scalar.sqrt(tmp_row[:], degrow[:])
            nc.vector.reciprocal(dinvrow[:], tmp_row[:])

            # dinv broadcast across partitions: rank-1 matmul ones x dinvrow
            dinvbc = cst.tile([128, NSH], F32, tag="dinvbc")
            psb0 = pfm.tile([128, NSH], F32, tag="fm")
            for o, n in SLICES:
                nc.tensor.matmul(
                    out=psb0[:, o : o + n], lhsT=ones_r[:, :],
                    rhs=dinvrow[:, o : o + n], start=True, stop=True,
                )
            nc.scalar.copy(dinvbc[:], psb0[:])

            def layer(inT, Ws, bs, h_tag):
                # W-matmul: P' rows for own nodes, node-major, bf16
                sh = wk.tile([128, NPAD], BF16, tag="sh")
                # zero the last chunk (covers the 30 pad rows); real rows are
                # overwritten by the j=9 activation below
                nc.vector.memset(sh[:, 9 * 128 : NPAD], 0.0)
                for j in range(NCH):
                    mj = 128 if j < 9 else MJ_LAST
                    pj = pnm.tile([128, H], F32, tag="nm")
                    nc.tensor.matmul(
                        out=pj[:mj, :], lhsT=inT[:, j * 128 : j * 128 + mj],
                        rhs=Ws[:], start=True, stop=False,
                    )
                    nc.tensor.matmul(
                        out=pj[:mj, :], lhsT=ones_r[:, :mj], rhs=bs[:],
                        start=False, stop=True,
                    )
                    nc.scalar.activation(
                        sh[:mj, j * 128 : (j + 1) * 128], pj[:mj, :],
                        AF.Copy, scale=dinvpp[:mj, j : j + 1],
                    )

                # AllGather shards, split in two halves so the A-matmul can
                # start on the first half while the second is in flight.
                # Half h of each rank's shard = local chunks j in [5h, 5h+5);
                # AllGather concatenates per-rank halves on the partition
                # axis, so half h's global chunk set is {k : 5h <= k%10 < 5h+5}.
                JH = NCH // 2            # 5 chunks per half
                RH = JH * 128            # 640 rows per half
                hps = []
                for hf in range(2):
                    bounce = dram.tile([RH, 128], BF16, tag=f"bounce{hf}")
                    agf = dram.tile([NFULL // 2, 128], BF16, tag=f"agf{hf}",
                                    addr_space="Shared")
                    nc.sync.dma_start(
                        out=bounce[:].rearrange("(j p) f -> p j f", p=128),
                        in_=sh[:, hf * RH : (hf + 1) * RH].rearrange(
                            "p (j f) -> p j f", f=128),
                    )
                    nc.gpsimd.collective_compute(
                        "AllGather", mybir.AluOpType.bypass, replica_groups=RG,
                        ins=[bounce.opt()], outs=[agf.opt()],
                    )
                    hp = hpp.tile([128, (KCH // 2) * 128], BF16, tag=f"hp{hf}")
                    nc.sync.dma_start(
                        out=hp[:].rearrange("p (q f) -> p q f", f=128),
                        in_=agf[:].rearrange("(q p) f -> p q f", p=128),
                    )
                    hps.append(hp)

                # A-matmul: Q^T[feat, dst] accumulated over 80 source chunks,
                # first-half chunks first (any order sums the same).
                ps = pfm.tile([128, NSH], F32, tag="fm")
                order = [(hf, c, j) for hf in range(2) for c in range(NC_)
                         for j in range(JH)]
                for i, (hf, c, j) in enumerate(order):
                    k = 10 * c + 5 * hf + j          # global source chunk
                    q = JH * c + j                   # chunk index within half
                    for o, n in SLICES:
                        nc.tensor.matmul(
                            out=ps[:, o : o + n],
                            lhsT=hps[hf][:, q * 128 : (q + 1) * 128],
                            rhs=asb[:, k * NSH + o : k * NSH + o + n],
                            start=(i == 0), stop=(i == KCH - 1),
                        )
                rel = wk.tile([128, NSH], F32, tag="rel")
                nc.scalar.activation(rel[:], ps[:], AF.Relu)
                hT = wk.tile([128, NSH], F32, tag=h_tag)
                nc.vector.tensor_mul(hT[:], rel[:], dinvbc[:])
                return hT

            h1T = layer(xT, W1s, b1s, "h1T")
            h2T = layer(h1T, W2s, b2s, "h2T")

            # ---- L2 normalize columns (per node) ----
            sq = wk.tile([128, NSH], F32, tag="sq")
            nc.scalar.square(sq[:], h2T[:])
            pns = pns_p.tile([1, NSH], F32, tag="ns")
            for o, n in SLICES:
                nc.tensor.matmul(
                    out=pns[:, o : o + n], lhsT=ones_c[:, :],
                    rhs=sq[:, o : o + n], start=True, stop=True,
                )
            sr = wk.tile([1, NSH], F32, tag="sr")
            nc.scalar.sqrt(sr[:], pns[:])
            nc.vector.tensor_scalar_max(sr[:], sr[:], 1e-12)
            rr = wk.tile([1, NSH], F32, tag="rr")
            nc.vector.reciprocal(rr[:], sr[:])
            psb = pfm.tile([128, NSH], F32, tag="fm")
            for o, n in SLICES:
                nc.tensor.matmul(
                    out=psb[:, o : o + n], lhsT=ones_r[:, :],
                    rhs=rr[:, o : o + n], start=True, stop=True,
                )
            hn = wk.tile([128, NSH], F32, tag="hn")
            nc.vector.tensor_mul(hn[:], h2T[:], psb[:])

            # ---- classifier ----
            oc = wk.tile([128, NCH * C], F32, tag="oc")
            nc.vector.memset(oc[:, 9 * C : NCH * C], 0.0)
            for j in range(NCH):
                mj = 128 if j < 9 else MJ_LAST
                pc = pnm.tile([128, H], F32, tag="nm")
                nc.tensor.matmul(
                    out=pc[:mj, :C], lhsT=hn[:, j * 128 : j * 128 + mj],
                    rhs=Wcs[:], start=True, stop=False,
                )
                nc.tensor.matmul(
                    out=pc[:mj, :C], lhsT=ones_r[:, :mj], rhs=bcs[:],
                    start=False, stop=True,
                )
                nc.scalar.copy(oc[:mj, j * C : (j + 1) * C], pc[:mj, :C])
            nc.sync.dma_start(
                out=out_d.rearrange("(j p) c -> p j c", p=128),
                in_=oc[:].rearrange("p (j c) -> p j c", c=C),
            )

    nc.compile()
    return nc


def _prep(inputs):
    x = np.asarray(inputs["x"], np.float32)
    ei = np.asarray(inputs["edge_index"])
    src = ei[0].astype(np.int64)
    dst = ei[1].astype(np.int64)
    loops = np.arange(N, dtype=np.int64)
    s_all = np.concatenate([src, loops])
    d_all = np.concatenate([dst, loops])
    deg = np.bincount(d_all, minlength=N).astype(np.float32)
    gsrc = (s_all // NSH) * NPAD + (s_all % NSH)  # padded global row of source
    dcore = d_all // NSH
    dloc = d_all % NSH

    W1 = np.ascontiguousarray(np.asarray(inputs["W1"], np.float32))
    W2 = np.ascontiguousarray(np.asarray(inputs["W2"], np.float32))
    Wc = np.ascontiguousarray(np.asarray(inputs["Wc"], np.float32))
    b1 = np.asarray(inputs["b1"], np.float32).reshape(1, H)
    b2 = np.asarray(inputs["b2"], np.float32).reshape(1, H)
    bc = np.asarray(inputs["bc"], np.float32).reshape(1, C)

    in_maps = []
    for c in range(NC_):
        m = dcore == c
        flat = gsrc[m] * NSH + dloc[m]
        Ac = np.bincount(flat, minlength=NFULL * NSH).astype(np.float32)
        Ac = Ac.reshape(KCH, 128, NSH).transpose(1, 0, 2)
        Ac = np.ascontiguousarray(Ac).reshape(128, KCH * NSH)
        degc = deg[c * NSH : (c + 1) * NSH]
        degpp = np.concatenate([degc, np.ones(NPAD - NSH, np.float32)])
        degpp = np.ascontiguousarray(degpp.reshape(NCH, 128).T)
        in_maps.append({
            "xT": np.ascontiguousarray(x[c * NSH : (c + 1) * NSH].T),
            "A": Ac.astype(ml_dtypes.float8_e4m3),
            "degpp": degpp,
            "degrow": np.ascontiguousarray(degc.reshape(1, NSH)),
            "W1": W1, "W2": W2, "Wc": Wc,
            "b1": b1, "b2": b2, "bc": bc,
        })
    return in_maps


def run(inputs, **spmd_kwargs):
    from concourse import bass_utils

    if "nc" not in _cache:
        _cache["nc"] = _build()
    in_maps = _prep(inputs)
    res = bass_utils.run_bass_kernel_spmd(
        _cache["nc"], in_maps, core_ids=list(range(NC_)), **spmd_kwargs
    )
    out = np.concatenate(
        [np.asarray(res.results[c]["out"])[:NSH] for c in range(NC_)], axis=0
    )
    return out.astype(np.float32), res


def kernel(**inputs):
    out, _ = run(inputs)
    return out
